# revision 8
# baseline (speedup 1.0000x reference)
"""Expert-parallel MoE routing kernel for 8 TRN2 NeuronCores.

softmax(relu(x @ W1[r] + b1[r]) @ W2[r] + b2[r]) per token, where r is the
token's route id.  Tokens are dispatched host-side (sorting by route is part
of sharding), one route per core; each core runs a padded two-layer MLP +
softmax in bf16 matmul / f32 accumulate.

Layer 2 streams TOKENS through stationary W2 blocks (cost proportional to
the actual token count, no 128-token-tile rounding): psum holds [v-block,
token] chunks, softmax row-sums come from a ones-matmul, the reciprocal is
partition-broadcast with a K=1 matmul, and the output lands [V, cap] in
DRAM (host transposes during the unshard scatter).
"""

import math

import numpy as np
import ml_dtypes

import concourse.bass as bass
import concourse.mybir as mybir
import concourse.tile as tile
from concourse import bacc
from concourse.bass_utils import run_bass_kernel_spmd

# Problem shape (nn_CategoryRouter): fixed by the grading harness.
B, S, D, F, V, R = 4, 1024, 768, 3072, 2048, 8
N_CORES = 8
KD = D // 128   # 6  K-tiles for layer 1
KF = F // 128   # 24 K-tiles for layer 2 (f blocks)
NVB = V // 128  # 16 128-wide v blocks
NQ = 4          # v blocks per pass (one DMA quad = [128, 512])

BF16 = mybir.dt.bfloat16
F32 = mybir.dt.float32
np_bf16 = ml_dtypes.bfloat16

_CACHE: dict[tuple, object] = {}


def _chunks(cap: int) -> list[tuple[int, int]]:
    """Split cap tokens into near-equal chunks of <=512 (psum bank width).

    Chunks of >=256 keep the per-matmul LDWEIGHTS (128 cols @1.2GHz)
    hidden under the moving-operand stream (N cols @2.4GHz).
    """
    n = max(1, math.ceil(cap / 512))
    base = cap // n
    sizes = [base + (1 if i < cap - base * n else 0) for i in range(n)]
    out, off = [], 0
    for s in sizes:
        out.append((off, s))
        off += s
    return out


def _build(cap: int, use_b2: bool):
    """One-core SPMD graph: [cap,D] tokens through its route's head."""
    AF = mybir.ActivationFunctionType
    chunks = _chunks(cap)

    nc = bacc.Bacc("TRN2", target_bir_lowering=False, debug=False,
                   num_devices=N_CORES)

    xt_d = nc.declare_dram_parameter("xt", [128, KD, cap], BF16, isOutput=False)
    w1_d = nc.declare_dram_parameter("w1", [KF, 128, KD * 128], BF16, isOutput=False)
    b1_d = nc.declare_dram_parameter("b1", [128, KF], F32, isOutput=False)
    # W2 stationary blocks, grouped in quads: [f, quad, 128 f-part, 4*128 v]
    w2_d = nc.declare_dram_parameter("w2", [KF, NQ, 128, NQ * 128], BF16,
                                     isOutput=False)
    b2_d = nc.declare_dram_parameter("b2", [1, V], BF16, isOutput=False)
    out_d = nc.declare_dram_parameter("out", [V, cap], F32, isOutput=True)

    with tile.TileContext(nc) as tc:
        with (
            tc.tile_pool(name="wpool", bufs=1) as wpool,
            tc.tile_pool(name="work", bufs=2) as work,
            tc.tile_pool(name="psum", bufs=8, space="PSUM") as psum,
        ):
            # Warm-up fodder: zeroed tiles for PE HAM ramp + Exp table load,
            # all runnable during the initial DMA fill.  memset pinned to
            # vector — gpsimd doesn't run its first instruction until ~6us
            # after NEFF start and would delay the PE ramp.
            wz = wpool.tile([128, 512], BF16, name="wz")
            nc.gpsimd.memset(wz[:], 0.0)
            dummy = work.tile([1, 2], F32, name="dummy", tag="dummy", bufs=1)
            nc.scalar.activation(dummy[:], wz[:1, :2], AF.Exp)
            ps_w = psum.tile([128, 512], F32, name="ps_w", tag="mm", bufs=6)
            n_warm = 8
            for i in range(n_warm):
                nc.tensor.matmul(ps_w[:], lhsT=wz[:, :128], rhs=wz[:],
                                 start=(i == 0), stop=(i == n_warm - 1))

            # Resident inputs, DMA'd once in consumption order over two DMA
            # rings (sync HWDGE + gpsimd SWDGE).  The scalar ring stays free:
            # scalar runs the psum exp-evictions and queued DMA triggers
            # there stall the pipeline.
            xt_s = [wpool.tile([128, cap], BF16, name=f"xt_s{k}", tag=f"xt_{k}")
                    for k in range(KD)]
            b1_s = wpool.tile([128, KF], F32, name="b1_s")
            w1_s = [wpool.tile([128, KD * 128], BF16, name=f"w1_s{f}",
                               tag=f"w1_{f}") for f in range(KF)]
            # W2 quads: w2q[f][q][:, j*128:(j+1)*128] is the stationary
            # block for v-block q*NQ+j.
            w2q = [[wpool.tile([128, NQ * 128], BF16, name=f"w2_{f}_{q}",
                               tag=f"w2_{f}_{q}") for q in range(NQ)]
                   for f in range(KF)]
            sync_q = [(xt_s[0], xt_d[:, 0, :]), (w1_s[0], w1_d[0]),
                      (xt_s[2], xt_d[:, 2, :]), (xt_s[4], xt_d[:, 4, :])] + \
                     [(w1_s[f], w1_d[f]) for f in range(2, KF, 2)]
            gp_q = [(xt_s[1], xt_d[:, 1, :]), (xt_s[3], xt_d[:, 3, :]),
                    (xt_s[5], xt_d[:, 5, :]), (b1_s, b1_d[:])] + \
                   [(w1_s[f], w1_d[f]) for f in range(1, KF, 2)]
            for i, (q, f) in enumerate((q, f) for q in range(NQ)
                                       for f in range(KF)):
                (sync_q if i % 2 == 0 else gp_q).append((w2q[f][q], w2_d[f, q]))
            for eng, qq in ((nc.sync, sync_q), (nc.gpsimd, gp_q)):
                for dst, src in qq:
                    eng.dma_start(out=dst[:], in_=src)

            ones_c = wpool.tile([128, 1], BF16, name="ones_c")   # col of ones
            ones_r = wpool.tile([1, 512], BF16, name="ones_r")   # row of ones
            nc.vector.memset(ones_c[:], 1.0)
            nc.vector.memset(ones_r[:], 1.0)
            b2_s = wpool.tile([1, V], BF16, name="b2_s")
            if use_b2:
                nc.sync.dma_start(out=b2_s[:], in_=b2_d[:])

            # Layer 1: ht[f] = relu(W1[:, f-block].T @ X.T + b1[f-block]),
            # stored [F-part, token] so it feeds layer 2 as the moving rhs.
            ht = [wpool.tile([128, cap], BF16, name=f"ht{f}", tag=f"ht_{f}")
                  for f in range(KF)]
            for f in range(KF):
                pss = [psum.tile([128, 512], F32, name=f"ps1_{f}_{c}", tag="mm",
                                 bufs=6) for c, _ in enumerate(chunks)]
                for k in range(KD):
                    for ps, (off, sz) in zip(pss, chunks):
                        nc.tensor.matmul(
                            ps[:, :sz],
                            lhsT=w1_s[f][:, k * 128:(k + 1) * 128],
                            rhs=xt_s[k][:, off:off + sz],
                            start=(k == 0), stop=(k == KD - 1),
                        )
                for ps, (off, sz) in zip(pss, chunks):
                    # relu(x+b1) on the (otherwise idle) vector engine keeps
                    # psum eviction latency off the scalar engine's queue.
                    nc.vector.tensor_scalar(
                        ht[f][:, off:off + sz], ps[:, :sz],
                        b1_s[:, f:f + 1], 0.0,
                        op0=mybir.AluOpType.add, op1=mybir.AluOpType.max)

            # Layer 2 + softmax, one chunk of W tokens at a time.
            # psum[v-block] = sum_f W2[f,v].T @ ht[f][:, chunk]  ([128v, W]).
            # Four passes of NQ v-blocks keep <=4 accumulating banks in
            # flight; exp evictions (scalar, bf16 out) free banks for the
            # next pass with ~500ns of slack per tile.
            for ci, (coff, W) in enumerate(chunks):
                exps = [work.tile([128, 512], BF16, name=f"exps{ci}_{v}",
                                  tag=f"exps_{v}", bufs=1) for v in range(NVB)]
                s1 = work.tile([128, 512], F32, name=f"s1_{ci}", tag="s1",
                               bufs=2)
                for q in range(NQ):
                    pss = [psum.tile([128, 512], F32, name=f"ps2_{ci}_{q}_{j}",
                                     tag="mm", bufs=6) for j in range(NQ)]
                    for f in range(KF):
                        for j in range(NQ):
                            nc.tensor.matmul(
                                pss[j][:, :W],
                                lhsT=w2q[f][q][:, j * 128:(j + 1) * 128],
                                rhs=ht[f][:, coff:coff + W],
                                start=(f == 0),
                                stop=(f == KF - 1 and not use_b2),
                            )
                    for j in range(NQ):
                        v = q * NQ + j
                        if use_b2:
                            # psum[v] += b2[v-block] broadcast over tokens.
                            nc.tensor.matmul(
                                pss[j][:, :W],
                                lhsT=b2_s[:, v * 128:(v + 1) * 128],
                                rhs=ones_r[:1, :W],
                                start=False, stop=True,
                            )
                        nc.scalar.activation(exps[v][:, :W], pss[j][:, :W],
                                             AF.Exp)
                        # Running v-sum on vector: s1 += exp_v.
                        if v == 1:
                            nc.vector.tensor_tensor(
                                s1[:, :W], exps[0][:, :W], exps[1][:, :W],
                                mybir.AluOpType.add)
                        elif v >= 2:
                            nc.vector.tensor_tensor(
                                s1[:, :W], s1[:, :W], exps[v][:, :W],
                                mybir.AluOpType.add)

                # Cross-partition row sum via ones-matmul, then reciprocal,
                # then K=1 broadcast matmul so the divide is elementwise.
                # Matmul operands must both be bf16: round the f32 running
                # sum and the reciprocal (common-mode ~0.2% per token, well
                # inside the 2e-2 gate).
                s1b = work.tile([128, 512], BF16, name=f"s1b_{ci}", tag="s1b",
                                bufs=2)
                nc.vector.tensor_copy(s1b[:, :W], s1[:, :W])
                ps_s = psum.tile([1, 512], F32, name=f"ps_s{ci}", tag="ps_s",
                                 bufs=1)
                nc.tensor.matmul(ps_s[:1, :W], lhsT=ones_c[:, :1],
                                 rhs=s1b[:, :W], start=True, stop=True)
                rec = work.tile([1, 512], BF16, name=f"rec{ci}", tag="rec",
                                bufs=2)
                with nc.allow_low_precision(
                        reason="softmax scale: bf16 reciprocal is a "
                               "common-mode ~0.2% per token, gate is 2e-2"):
                    nc.vector.reciprocal(rec[:1, :W], ps_s[:1, :W])
                ps_b = psum.tile([128, 512], F32, name=f"ps_b{ci}", tag="ps_b",
                                 bufs=1)
                nc.tensor.matmul(ps_b[:, :W], lhsT=ones_r[:1, :128],
                                 rhs=rec[:1, :W], start=True, stop=True)
                for v in range(NVB):
                    ot = work.tile([128, 512], F32, name=f"ot{ci}_{v}",
                                   tag="ot", bufs=4)
                    nc.vector.tensor_tensor(ot[:, :W], exps[v][:, :W],
                                            ps_b[:, :W],
                                            mybir.AluOpType.mult)
                    [nc.sync, nc.gpsimd][v % 2].dma_start(
                        out=out_d[v * 128:(v + 1) * 128, coff:coff + W],
                        in_=ot[:, :W])

    nc.compile()
    return nc


def _dispatch(e_two, route_ids, W1, b1, W2, b2):
    """Host-side shard: sort tokens by route, pad, tile weights per core."""
    x = np.ascontiguousarray(e_two, dtype=np.float32).reshape(-1, D)
    rid = np.asarray(route_ids).reshape(-1)
    order = np.argsort(rid, kind="stable")
    counts = np.bincount(rid, minlength=R)
    cap = max(256, int(math.ceil(counts.max() / 16)) * 16)

    in_maps, perms = [], []
    start = 0
    for r in range(R):
        n = int(counts[r])
        toks = order[start:start + n]
        start += n
        perms.append(toks)

        xp = np.zeros((cap, D), np.float32)
        xp[:n] = x[toks]
        # [128, KD, cap]: partition p holds feature k*128+p of every token.
        xt = np.ascontiguousarray(
            xp.T.reshape(KD, 128, cap).transpose(1, 0, 2)).astype(np_bf16)
        # [KF, 128, KD*128]: row p of block f holds W1[k*128+p, f*128+m].
        w1 = np.ascontiguousarray(
            np.asarray(W1[r], np.float32).reshape(KD, 128, KF, 128)
            .transpose(2, 1, 0, 3).reshape(KF, 128, KD * 128)).astype(np_bf16)
        b1t = np.ascontiguousarray(
            np.asarray(b1[r], np.float32).reshape(KF, 128).T)
        # [KF, NQ, 128, NQ*128]: quad q row p col j*128+c holds
        # W2[f*128+p, (q*NQ+j)*128+c].
        w2 = np.ascontiguousarray(
            np.asarray(W2[r], np.float32)
            .reshape(KF, 128, NQ, NQ, 128)
            .transpose(0, 2, 1, 3, 4)
            .reshape(KF, NQ, 128, NQ * 128)).astype(np_bf16)
        b2t = np.asarray(b2[r], np.float32).reshape(1, V).astype(np_bf16)
        in_maps.append({"xt": xt, "w1": w1, "b1": b1t, "w2": w2, "b2": b2t})
    return in_maps, perms, counts, cap


def kernel(e_two, route_ids, W1, b1, W2, b2):
    in_maps, perms, counts, cap = _dispatch(e_two, route_ids, W1, b1, W2, b2)
    use_b2 = bool(np.any(np.asarray(b2)))

    key = (cap, use_b2)
    nc = _CACHE.get(key)
    if nc is None:
        nc = _build(cap, use_b2)
        _CACHE[key] = nc

    res = run_bass_kernel_spmd(nc, in_maps, core_ids=list(range(N_CORES)))

    out = np.zeros((B * S, V), np.float32)
    for r in range(R):
        out[perms[r]] = res.results[r]["out"][:, :counts[r]].T
    return out.reshape(B, S, V)
